# revision 14
# baseline (speedup 1.0000x reference)
"""Trainium2 Bass kernel for nn_ConvBundle_48146583388363.

Math: out[x,y,b,i,j,o] = s[b, i+x-1, j+y-1] * wsum[x,y,o]
  where s = inputs.sum(channel) (zero-padded at borders) and
  wsum = W.sum(axis=2).

Sharding: data-parallel over batch B=16 across 8 cores (2 batches/core).

Layout: cout (o=128) on SBUF partitions, flat per-batch spatial index
f = 96*i + j on the free dim. One PE matmul (ones[64,128].T @ x[64,f])
does the channel reduce AND broadcasts s[f] to all 128 partitions.
Each tap's shift is then just a free-dim AP offset into an s buffer
with 98-wide zero halos; each (tap, batch, half) output is a single
[128, 4608] tensor_scalar_mul with the per-partition scalar wsum[o],
which hits the DVE 4x perf mode (fp16, step-1, 4B-aligned, SBUF).

Two s copies at both alignment parities (s_e[98+f]=s[f] on DVE,
s_o[99+f]=s[f] on ACT, both cast f32->f16 from PSUM) make every tap
offset even so the 4x mode alignment requirement holds for all 9 taps.
Column-border zeros for the dy!=0 taps are strided memsets on the
output slab before DMA.

Output is written fp16 (halves the HBM write traffic, which is the
roofline: ~45 MB/core at ~358 GB/s). Host upcasts to f32 during
unshard; rel err ~1e-3 vs the 2e-2 gate.
"""

import ml_dtypes
import numpy as np

import concourse.bacc as bacc
import concourse.mybir as mybir
from concourse import tile
from concourse.bass_utils import run_bass_kernel_spmd

F32 = mybir.dt.float32
F16 = mybir.dt.float16
BF16 = mybir.dt.bfloat16

NCORES = 8
B, H, W_, CIN = 16, 96, 96, 64
COUT = 128
BPC = B // NCORES          # batches per core = 2
SP = H * W_                # 9216 spatial positions per batch
NTAP = 9
TAPS = [(x - 1, y - 1) for x in range(3) for y in range(3)]  # tap n = 3x+y
CK = 512                   # PSUM chunk (one bank) in f
NCK = SP // CK             # 18 chunks per batch
HALO = 98
L = HALO + SP + HALO       # s buffer length = 9412
HB = SP // 2               # money-op granularity = 4608


def _build_nc():
    nc = bacc.Bacc(None, target_bir_lowering=False)
    x = nc.dram_tensor("x", [BPC, CIN, SP], BF16, kind="ExternalInput")
    w = nc.dram_tensor("w", [COUT, NTAP * COUT], BF16, kind="ExternalInput")
    y = nc.dram_tensor("y", [NTAP, BPC, COUT, SP], F16, kind="ExternalOutput")

    with tile.TileContext(nc) as tc:
        with (
            tc.tile_pool(name="const", bufs=1) as cpool,
            tc.tile_pool(name="xin", bufs=1) as xpool,
            tc.tile_pool(name="psum_s", bufs=7, space="PSUM") as pspool,
            tc.tile_pool(name="psum_w", bufs=1, space="PSUM") as pwpool,
            tc.tile_pool(name="out", bufs=5) as opool,
        ):
            # --- input DMAs. W first (it gates wsum -> every money op);
            # batch 0 quartered across both HWDGE rings for an early start.
            w_sb = cpool.tile([COUT, NTAP * COUT], BF16, name="w_sb")
            nc.scalar.dma_start(out=w_sb[:], in_=w[:])
            xts = [xpool.tile([CIN, SP], BF16, name=f"xt{b}") for b in range(BPC)]
            QW = SP // 4
            for q, eng in enumerate([nc.sync, nc.scalar, nc.sync, nc.scalar]):
                eng.dma_start(
                    out=xts[0][:, q * QW:(q + 1) * QW], in_=x[0][:, q * QW:(q + 1) * QW]
                )
            for hlf in range(2):
                nc.sync.dma_start(
                    out=xts[1][:, hlf * HB:(hlf + 1) * HB],
                    in_=x[1][:, hlf * HB:(hlf + 1) * HB],
                )

            # --- constants (no DMA: generated on-chip)
            ones64 = cpool.tile([CIN, COUT], BF16, name="ones64")
            nc.vector.memset(ones64[:], 1.0)
            onesc = cpool.tile([COUT, 1], BF16, name="onesc")
            nc.vector.memset(onesc[:], 1.0)

            # s buffers, both parities, zero halos
            s_e, s_o = [], []
            for b in range(BPC):
                se = cpool.tile([COUT, L], F16, name=f"s_e{b}")
                so = cpool.tile([COUT, L], F16, name=f"s_o{b}")
                nc.vector.memset(se[:, 0:HALO], 0.0)
                nc.vector.memset(se[:, HALO + SP:L], 0.0)
                nc.vector.memset(so[:, 0:HALO + 1], 0.0)
                nc.vector.memset(so[:, HALO + 1 + SP:L], 0.0)
                s_e.append(se)
                s_o.append(so)

            # --- wsum[o, n] = sum_c W[n, c, o]: 9 single-column matmuls.
            # ws lands via DVE (not ACT) so the s_o copy stream is never
            # head-of-line blocked behind the W dependency chain.
            pw = pwpool.tile([COUT, CK], F32, name="pw")
            for n in range(NTAP):
                nc.tensor.matmul(
                    pw[:, n:n + 1],
                    lhsT=w_sb[:, n * COUT:(n + 1) * COUT],
                    rhs=onesc[:],
                    start=True, stop=True, skip_group_check=True,
                )
            ws = cpool.tile([COUT, 16], F32, name="ws")
            nc.vector.tensor_copy(ws[:, 0:NTAP], pw[:, 0:NTAP])

            # tap read offsets (all even by construction)
            def tap_src(b, n, f0, fd):
                dx, dy = TAPS[n]
                d = 96 * dx + dy
                if dy == 0:
                    buf, st = s_e[b], HALO + d
                else:
                    buf, st = s_o[b], HALO + 1 + d
                return buf[:, st + f0: st + f0 + fd]

            slab_seq = []  # emission counter for DMA ring alternation

            def emit_money(b, f0, fd, sbufs=3):
                """One money op + border fix + DMA per tap for f range
                [f0, f0+fd). The slab tile covers exactly that range."""
                # dy==0 taps first: they only need s_e (DVE) chunks, which
                # land earlier than the ACT s_o chunks.
                for n in sorted(range(NTAP), key=lambda n: TAPS[n][1] != 0):
                    dx, dy = TAPS[n]
                    slab = opool.tile(
                        [COUT, fd], F16, name=f"slab{n}_{b}_{f0}",
                        tag=f"slab{fd}", bufs=sbufs,
                    )
                    nc.vector.tensor_scalar_mul(
                        slab[:], tap_src(b, n, f0, fd), ws[:, n:n + 1]
                    )
                    if dy != 0:
                        sv = slab[:].rearrange("p (i j) -> p i j", j=96)
                        jz = 0 if dy < 0 else 95
                        nc.vector.memset(sv[:, :, jz:jz + 1], 0.0)
                    slab_seq.append(n)
                    eng = nc.sync if len(slab_seq) % 2 else nc.scalar
                    eng.dma_start(
                        out=y[n, b][:, f0:f0 + fd], in_=slab[:]
                    )

            tap_order = sorted(range(NTAP), key=lambda n: TAPS[n][1] != 0)

            def emit_full_h0(b):
                """Whole-batch slabs [COUT, SP] (-> one 2.25MB DMA with
                18.4KB/partition contiguous runs): first halves only."""
                slabs = {}
                for n in tap_order:
                    slabs[n] = opool.tile(
                        [COUT, SP], F16, name=f"fslab{n}_{b}",
                        tag="fslab", bufs=2,
                    )
                    nc.vector.tensor_scalar_mul(
                        slabs[n][:, 0:HB], tap_src(b, n, 0, HB), ws[:, n:n + 1]
                    )
                return slabs

            def emit_full_h1(b, slabs):
                for n in tap_order:
                    dx, dy = TAPS[n]
                    slab = slabs[n]
                    nc.vector.tensor_scalar_mul(
                        slab[:, HB:SP], tap_src(b, n, HB, HB), ws[:, n:n + 1]
                    )
                    if dy != 0:
                        sv = slab[:].rearrange("p (i j) -> p i j", j=96)
                        jz = 0 if dy < 0 else 95
                        nc.vector.memset(sv[:, :, jz:jz + 1], 0.0)
                    slab_seq.append(n)
                    eng = nc.sync if len(slab_seq) % 2 else nc.scalar
                    eng.dma_start(out=y[n, b][:], in_=slab[:])

            # --- main pipeline
            fslabs = None
            for b in range(BPC):
                for k in range(NCK):
                    ps = pspool.tile([COUT, CK], F32, name=f"ps{b}_{k}", tag="ps")
                    nc.tensor.matmul(
                        ps[:], lhsT=ones64[:],
                        rhs=xts[b][:, k * CK:(k + 1) * CK],
                        start=True, stop=True,
                    )
                    nc.vector.tensor_copy(
                        s_e[b][:, HALO + k * CK:HALO + (k + 1) * CK], ps[:]
                    )
                    nc.scalar.copy(
                        s_o[b][:, HALO + 1 + k * CK:HALO + 1 + (k + 1) * CK], ps[:]
                    )
                    # each piece is emitted only after every cast it reads
                    # is already in the DVE stream (no reliance on the
                    # scheduler hoisting casts past blocked money ops).
                    # batch 0: small early pieces so output DMA starts ASAP;
                    # batch 1: full-batch slabs for max DMA run length.
                    if b == 0 and k == 4:
                        emit_money(b, 0, HB // 2)
                    elif b == 0 and k == 9:
                        emit_money(b, HB // 2, HB // 2)
                    elif b == 1 and k == 9:
                        fslabs = emit_full_h0(b)
                if b == 0:
                    emit_money(b, HB, HB)
                else:
                    emit_full_h1(b, fslabs)

    nc.finalize()
    return nc


_CACHE = {}


def _get_nc():
    if "nc" not in _CACHE:
        _CACHE["nc"] = _build_nc()
    return _CACHE["nc"]


def _run(x_full, w_full, **kwargs):
    nc = _get_nc()
    # W[n, c, o] -> [c, n*o] so the colsum matmul's lhsT ([c, o] slices) is
    # a plain contiguous SBUF tile.
    wt = np.ascontiguousarray(
        w_full.reshape(NTAP, COUT, COUT).transpose(1, 0, 2)
    ).reshape(COUT, NTAP * COUT).astype(ml_dtypes.bfloat16)
    # per core: [BPC, 9216, 64] -> [BPC, 64, 9216] bf16 (channel-major so the
    # PE ones-matmul contracts over the partition dim)
    xr = x_full.reshape(NCORES, BPC, SP, CIN)
    in_maps = [
        {
            "x": np.ascontiguousarray(
                xr[c].transpose(0, 2, 1).astype(ml_dtypes.bfloat16)
            ),
            "w": wt,
        }
        for c in range(NCORES)
    ]
    return run_bass_kernel_spmd(nc, in_maps, core_ids=list(range(NCORES)), **kwargs)


def _unshard(results):
    """Per-core y is [9, BPC, 128(o), 9216(f)]; full out wants [..., f, o]."""
    out = np.empty((3, 3, B, H, W_, COUT), np.float32)
    ov = out.reshape(NTAP, B, SP, COUT)
    for c, r in enumerate(results):
        yc = np.asarray(r["y"]).reshape(NTAP, BPC, COUT, SP)
        ov[:, BPC * c:BPC * (c + 1)] = yc.transpose(0, 1, 3, 2)
    return out


def kernel(**inputs):
    x_full = np.ascontiguousarray(np.asarray(inputs["inputs"], dtype=np.float32))
    w_full = np.ascontiguousarray(np.asarray(inputs["W"], dtype=np.float32))
    res = _run(x_full, w_full)
    return _unshard(res.results)


# revision 16
# speedup vs baseline: 1.0992x; 1.0992x over previous
"""Trainium2 Bass kernel for nn_ConvBundle_48146583388363.

Math: out[x,y,b,i,j,o] = s[b, i+x-1, j+y-1] * wsum[x,y,o]
  where s = inputs.sum(channel) (zero-padded at borders) and
  wsum = W.sum(axis=2).

Sharding: data-parallel over batch B=16 across 8 cores (2 batches/core).

Layout: cout (o=128) on SBUF partitions, flat per-batch spatial index
f = 96*i + j on the free dim. One PE matmul (ones[64,128].T @ x[64,f])
does the channel reduce AND broadcasts s[f] to all 128 partitions.
Each tap's shift is then just a free-dim AP offset into an s buffer
with 98-wide zero halos; each (tap, batch, half) output is a single
[128, 4608] tensor_scalar_mul with the per-partition scalar wsum[o],
which hits the DVE 4x perf mode (fp16, step-1, 4B-aligned, SBUF).

Two s copies at both alignment parities (s_e[98+f]=s[f] on DVE,
s_o[99+f]=s[f] on ACT, both cast f32->f16 from PSUM) make every tap
offset even so the 4x mode alignment requirement holds for all 9 taps.
Column-border zeros for the dy!=0 taps are strided memsets on the
output slab before DMA.

Output is written fp16 (halves the HBM write traffic, which is the
roofline: ~45 MB/core at ~358 GB/s). Host upcasts to f32 during
unshard; rel err ~1e-3 vs the 2e-2 gate.
"""

import ml_dtypes
import numpy as np

import concourse.bacc as bacc
import concourse.mybir as mybir
from concourse import tile
from concourse.bass_utils import run_bass_kernel_spmd

F32 = mybir.dt.float32
F16 = mybir.dt.float16
BF16 = mybir.dt.bfloat16

NCORES = 8
B, H, W_, CIN = 16, 96, 96, 64
COUT = 128
BPC = B // NCORES          # batches per core = 2
SP = H * W_                # 9216 spatial positions per batch
NTAP = 9
TAPS = [(x - 1, y - 1) for x in range(3) for y in range(3)]  # tap n = 3x+y
CK = 512                   # PSUM chunk (one bank) in f
NCK = SP // CK             # 18 chunks per batch
HALO = 98
L = HALO + SP + HALO       # s buffer length = 9412
HB = SP // 2               # money-op granularity = 4608


def _build_nc():
    nc = bacc.Bacc(None, target_bir_lowering=False)
    x = nc.dram_tensor("x", [BPC, CIN, SP], BF16, kind="ExternalInput")
    w = nc.dram_tensor("w", [COUT, NTAP * COUT], BF16, kind="ExternalInput")
    y = nc.dram_tensor("y", [NTAP, BPC, COUT, SP], F16, kind="ExternalOutput")

    with tile.TileContext(nc) as tc:
        with (
            tc.tile_pool(name="const", bufs=1) as cpool,
            tc.tile_pool(name="xin", bufs=1) as xpool,
            tc.tile_pool(name="psum_s", bufs=7, space="PSUM") as pspool,
            tc.tile_pool(name="psum_w", bufs=1, space="PSUM") as pwpool,
            tc.tile_pool(name="out", bufs=5) as opool,
        ):
            # --- input DMAs. W first (it gates wsum -> every money op);
            # batch 0 quartered across both HWDGE rings for an early start.
            w_sb = cpool.tile([COUT, NTAP * COUT], BF16, name="w_sb")
            nc.scalar.dma_start(out=w_sb[:], in_=w[:])
            xts = [xpool.tile([CIN, SP], BF16, name=f"xt{b}") for b in range(BPC)]
            QW = SP // 4
            for q, eng in enumerate([nc.sync, nc.scalar, nc.sync, nc.scalar]):
                eng.dma_start(
                    out=xts[0][:, q * QW:(q + 1) * QW], in_=x[0][:, q * QW:(q + 1) * QW]
                )
            for hlf in range(2):
                nc.sync.dma_start(
                    out=xts[1][:, hlf * HB:(hlf + 1) * HB],
                    in_=x[1][:, hlf * HB:(hlf + 1) * HB],
                )

            # --- constants (no DMA: generated on-chip)
            ones64 = cpool.tile([CIN, COUT], BF16, name="ones64")
            nc.vector.memset(ones64[:], 1.0)
            onesc = cpool.tile([COUT, 1], BF16, name="onesc")
            nc.vector.memset(onesc[:], 1.0)

            # s buffers, both parities, zero halos
            s_e, s_o = [], []
            for b in range(BPC):
                se = cpool.tile([COUT, L], F16, name=f"s_e{b}")
                so = cpool.tile([COUT, L], F16, name=f"s_o{b}")
                nc.vector.memset(se[:, 0:HALO], 0.0)
                nc.vector.memset(se[:, HALO + SP:L], 0.0)
                nc.vector.memset(so[:, 0:HALO + 1], 0.0)
                nc.vector.memset(so[:, HALO + 1 + SP:L], 0.0)
                s_e.append(se)
                s_o.append(so)

            # --- wsum[o, n] = sum_c W[n, c, o]: 9 single-column matmuls.
            # ws lands via DVE (not ACT) so the s_o copy stream is never
            # head-of-line blocked behind the W dependency chain.
            pw = pwpool.tile([COUT, CK], F32, name="pw")
            for n in range(NTAP):
                nc.tensor.matmul(
                    pw[:, n:n + 1],
                    lhsT=w_sb[:, n * COUT:(n + 1) * COUT],
                    rhs=onesc[:],
                    start=True, stop=True, skip_group_check=True,
                )
            ws = cpool.tile([COUT, 16], F32, name="ws")
            nc.vector.tensor_copy(ws[:, 0:NTAP], pw[:, 0:NTAP])

            # tap read offsets (all even by construction)
            def tap_src(b, n, f0, fd):
                dx, dy = TAPS[n]
                d = 96 * dx + dy
                if dy == 0:
                    buf, st = s_e[b], HALO + d
                else:
                    buf, st = s_o[b], HALO + 1 + d
                return buf[:, st + f0: st + f0 + fd]

            slab_seq = []  # emission counter for DMA ring alternation

            def emit_money(b, f0, fd, sbufs=3):
                """One money op + border fix + DMA per tap for f range
                [f0, f0+fd). The slab tile covers exactly that range."""
                # dy==0 taps first: they only need s_e (DVE) chunks, which
                # land earlier than the ACT s_o chunks.
                for n in sorted(range(NTAP), key=lambda n: TAPS[n][1] != 0):
                    dx, dy = TAPS[n]
                    slab = opool.tile(
                        [COUT, fd], F16, name=f"slab{n}_{b}_{f0}",
                        tag=f"slab{fd}", bufs=sbufs,
                    )
                    nc.vector.tensor_scalar_mul(
                        slab[:], tap_src(b, n, f0, fd), ws[:, n:n + 1]
                    )
                    if dy != 0:
                        sv = slab[:].rearrange("p (i j) -> p i j", j=96)
                        jz = 0 if dy < 0 else 95
                        nc.vector.memset(sv[:, :, jz:jz + 1], 0.0)
                    slab_seq.append(n)
                    eng = nc.sync if len(slab_seq) % 2 else nc.scalar
                    eng.dma_start(
                        out=y[n, b][:, f0:f0 + fd], in_=slab[:]
                    )

            tap_order = sorted(range(NTAP), key=lambda n: TAPS[n][1] != 0)

            def emit_full(b):
                """Whole-batch money per tap [COUT, SP] -> one 2.25MB DMA
                with 18.4KB/partition contiguous runs. Emitted after all of
                batch b's casts; steady produce->drain keeps DMA fed."""
                for n in tap_order:
                    dx, dy = TAPS[n]
                    slab = opool.tile(
                        [COUT, SP], F16, name=f"fslab{n}_{b}",
                        tag="fslab", bufs=2,
                    )
                    nc.vector.tensor_scalar_mul(
                        slab[:], tap_src(b, n, 0, SP), ws[:, n:n + 1]
                    )
                    if dy != 0:
                        sv = slab[:].rearrange("p (i j) -> p i j", j=96)
                        jz = 0 if dy < 0 else 95
                        nc.vector.memset(sv[:, :, jz:jz + 1], 0.0)
                    slab_seq.append(n)
                    eng = nc.sync if len(slab_seq) % 2 else nc.scalar
                    eng.dma_start(out=y[n, b][:], in_=slab[:])

            # --- main pipeline
            for b in range(BPC):
                for k in range(NCK):
                    ps = pspool.tile([COUT, CK], F32, name=f"ps{b}_{k}", tag="ps")
                    nc.tensor.matmul(
                        ps[:], lhsT=ones64[:],
                        rhs=xts[b][:, k * CK:(k + 1) * CK],
                        start=True, stop=True,
                    )
                    nc.vector.tensor_copy(
                        s_e[b][:, HALO + k * CK:HALO + (k + 1) * CK], ps[:]
                    )
                    nc.scalar.copy(
                        s_o[b][:, HALO + 1 + k * CK:HALO + 1 + (k + 1) * CK], ps[:]
                    )
                    # each piece is emitted only after every cast it reads
                    # is already in the DVE stream (no reliance on the
                    # scheduler hoisting casts past blocked money ops).
                    # batch 0: small early pieces so output DMA starts ASAP;
                    # batch 1: full-batch slabs for max DMA run length.
                    if b == 0 and k == 4:
                        emit_money(b, 0, HB // 2)
                    elif b == 0 and k == 9:
                        emit_money(b, HB // 2, HB // 2)
                if b == 0:
                    emit_money(b, HB, HB, sbufs=4)
                else:
                    emit_full(b)

    nc.finalize()
    return nc


_CACHE = {}


def _get_nc():
    if "nc" not in _CACHE:
        _CACHE["nc"] = _build_nc()
    return _CACHE["nc"]


def _run(x_full, w_full, **kwargs):
    nc = _get_nc()
    # W[n, c, o] -> [c, n*o] so the colsum matmul's lhsT ([c, o] slices) is
    # a plain contiguous SBUF tile.
    wt = np.ascontiguousarray(
        w_full.reshape(NTAP, COUT, COUT).transpose(1, 0, 2)
    ).reshape(COUT, NTAP * COUT).astype(ml_dtypes.bfloat16)
    # per core: [BPC, 9216, 64] -> [BPC, 64, 9216] bf16 (channel-major so the
    # PE ones-matmul contracts over the partition dim)
    xr = x_full.reshape(NCORES, BPC, SP, CIN)
    in_maps = [
        {
            "x": np.ascontiguousarray(
                xr[c].transpose(0, 2, 1).astype(ml_dtypes.bfloat16)
            ),
            "w": wt,
        }
        for c in range(NCORES)
    ]
    return run_bass_kernel_spmd(nc, in_maps, core_ids=list(range(NCORES)), **kwargs)


def _unshard(results):
    """Per-core y is [9, BPC, 128(o), 9216(f)]; full out wants [..., f, o]."""
    out = np.empty((3, 3, B, H, W_, COUT), np.float32)
    ov = out.reshape(NTAP, B, SP, COUT)
    for c, r in enumerate(results):
        yc = np.asarray(r["y"]).reshape(NTAP, BPC, COUT, SP)
        ov[:, BPC * c:BPC * (c + 1)] = yc.transpose(0, 1, 3, 2)
    return out


def kernel(**inputs):
    x_full = np.ascontiguousarray(np.asarray(inputs["inputs"], dtype=np.float32))
    w_full = np.ascontiguousarray(np.asarray(inputs["W"], dtype=np.float32))
    res = _run(x_full, w_full)
    return _unshard(res.results)


# revision 19
# speedup vs baseline: 1.1013x; 1.0019x over previous
"""Trainium2 Bass kernel for nn_ConvBundle_48146583388363.

Math: out[x,y,b,i,j,o] = s[b, i+x-1, j+y-1] * wsum[x,y,o]
  where s = inputs.sum(channel) (zero-padded at borders) and
  wsum = W.sum(axis=2).

Sharding: data-parallel over batch B=16 across 8 cores (2 batches/core).

Layout: cout (o=128) on SBUF partitions, flat per-batch spatial index
f = 96*i + j on the free dim. One PE matmul (ones[64,128].T @ x[64,f])
does the channel reduce AND broadcasts s[f] to all 128 partitions.
Each tap's shift is then just a free-dim AP offset into an s buffer
with 98-wide zero halos; each (tap, batch, half) output is a single
[128, 4608] tensor_scalar_mul with the per-partition scalar wsum[o],
which hits the DVE 4x perf mode (fp16, step-1, 4B-aligned, SBUF).

Two s copies at both alignment parities (s_e[98+f]=s[f] on DVE,
s_o[99+f]=s[f] on ACT, both cast f32->f16 from PSUM) make every tap
offset even so the 4x mode alignment requirement holds for all 9 taps.
Column-border zeros for the dy!=0 taps are strided memsets on the
output slab before DMA.

Output is written fp16 (halves the HBM write traffic, which is the
roofline: ~45 MB/core at ~358 GB/s). Host upcasts to f32 during
unshard; rel err ~1e-3 vs the 2e-2 gate.
"""

import ml_dtypes
import numpy as np

import concourse.bacc as bacc
import concourse.mybir as mybir
from concourse import tile
from concourse.bass_utils import run_bass_kernel_spmd

F32 = mybir.dt.float32
F16 = mybir.dt.float16
BF16 = mybir.dt.bfloat16

NCORES = 8
B, H, W_, CIN = 16, 96, 96, 64
COUT = 128
BPC = B // NCORES          # batches per core = 2
SP = H * W_                # 9216 spatial positions per batch
NTAP = 9
TAPS = [(x - 1, y - 1) for x in range(3) for y in range(3)]  # tap n = 3x+y
CK = 512                   # PSUM chunk (one bank) in f
NCK = SP // CK             # 18 chunks per batch
HALO = 98
L = HALO + SP + HALO       # s buffer length = 9412
HB = SP // 2               # money-op granularity = 4608


def _build_nc():
    nc = bacc.Bacc(None, target_bir_lowering=False)
    x = nc.dram_tensor("x", [BPC, CIN, SP], BF16, kind="ExternalInput")
    w = nc.dram_tensor("w", [COUT, NTAP * COUT], BF16, kind="ExternalInput")
    y = nc.dram_tensor("y", [NTAP, BPC, COUT, SP], F16, kind="ExternalOutput")

    with tile.TileContext(nc) as tc:
        with (
            tc.tile_pool(name="const", bufs=1) as cpool,
            tc.tile_pool(name="xin", bufs=1) as xpool,
            tc.tile_pool(name="psum_s", bufs=7, space="PSUM") as pspool,
            tc.tile_pool(name="psum_w", bufs=1, space="PSUM") as pwpool,
            tc.tile_pool(name="out", bufs=5) as opool,
        ):
            # --- input DMAs. W first (it gates wsum -> every money op);
            # batch 0 quartered across both HWDGE rings for an early start.
            w_sb = cpool.tile([COUT, NTAP * COUT], BF16, name="w_sb")
            nc.scalar.dma_start(out=w_sb[:], in_=w[:])
            xts = [xpool.tile([CIN, SP], BF16, name=f"xt{b}") for b in range(BPC)]
            QW = SP // 4
            for q, eng in enumerate([nc.sync, nc.scalar, nc.sync, nc.scalar]):
                eng.dma_start(
                    out=xts[0][:, q * QW:(q + 1) * QW], in_=x[0][:, q * QW:(q + 1) * QW]
                )
            for hlf, eng in enumerate([nc.sync, nc.scalar]):
                eng.dma_start(
                    out=xts[1][:, hlf * HB:(hlf + 1) * HB],
                    in_=x[1][:, hlf * HB:(hlf + 1) * HB],
                )

            # --- constants (no DMA: generated on-chip)
            ones64 = cpool.tile([CIN, COUT], BF16, name="ones64")
            nc.vector.memset(ones64[:], 1.0)
            onesc = cpool.tile([COUT, 1], BF16, name="onesc")
            nc.vector.memset(onesc[:], 1.0)

            # s buffers, both parities, zero halos
            s_e, s_o = [], []
            for b in range(BPC):
                se = cpool.tile([COUT, L], F16, name=f"s_e{b}")
                so = cpool.tile([COUT, L], F16, name=f"s_o{b}")
                nc.vector.memset(se[:, 0:HALO], 0.0)
                nc.vector.memset(se[:, HALO + SP:L], 0.0)
                nc.vector.memset(so[:, 0:HALO + 1], 0.0)
                nc.vector.memset(so[:, HALO + 1 + SP:L], 0.0)
                s_e.append(se)
                s_o.append(so)

            # --- wsum[o, n] = sum_c W[n, c, o]: 9 single-column matmuls.
            # ws lands via DVE (not ACT) so the s_o copy stream is never
            # head-of-line blocked behind the W dependency chain.
            pw = pwpool.tile([COUT, CK], F32, name="pw")
            for n in range(NTAP):
                nc.tensor.matmul(
                    pw[:, n:n + 1],
                    lhsT=w_sb[:, n * COUT:(n + 1) * COUT],
                    rhs=onesc[:],
                    start=True, stop=True, skip_group_check=True,
                )
            ws = cpool.tile([COUT, 16], F32, name="ws")
            nc.vector.tensor_copy(ws[:, 0:NTAP], pw[:, 0:NTAP])

            # tap read offsets (all even by construction)
            def tap_src(b, n, f0, fd):
                dx, dy = TAPS[n]
                d = 96 * dx + dy
                if dy == 0:
                    buf, st = s_e[b], HALO + d
                else:
                    buf, st = s_o[b], HALO + 1 + d
                return buf[:, st + f0: st + f0 + fd]

            slab_seq = []  # emission counter for DMA ring alternation

            def emit_money(b, f0, fd, sbufs=3):
                """One money op + border fix + DMA per tap for f range
                [f0, f0+fd). The slab tile covers exactly that range."""
                # dy==0 taps first: they only need s_e (DVE) chunks, which
                # land earlier than the ACT s_o chunks.
                for n in sorted(range(NTAP), key=lambda n: TAPS[n][1] != 0):
                    dx, dy = TAPS[n]
                    slab = opool.tile(
                        [COUT, fd], F16, name=f"slab{n}_{b}_{f0}",
                        tag=f"slab{fd}", bufs=sbufs,
                    )
                    nc.vector.tensor_scalar_mul(
                        slab[:], tap_src(b, n, f0, fd), ws[:, n:n + 1]
                    )
                    if dy != 0:
                        sv = slab[:].rearrange("p (i j) -> p i j", j=96)
                        jz = 0 if dy < 0 else 95
                        nc.vector.memset(sv[:, :, jz:jz + 1], 0.0)
                    slab_seq.append(n)
                    eng = nc.sync if len(slab_seq) % 2 else nc.scalar
                    eng.dma_start(
                        out=y[n, b][:, f0:f0 + fd], in_=slab[:]
                    )

            tap_order = sorted(range(NTAP), key=lambda n: TAPS[n][1] != 0)

            def emit_full(b):
                """Whole-batch money per tap [COUT, SP] -> one 2.25MB DMA
                with 18.4KB/partition contiguous runs. Emitted after all of
                batch b's casts; steady produce->drain keeps DMA fed."""
                for n in tap_order:
                    dx, dy = TAPS[n]
                    slab = opool.tile(
                        [COUT, SP], F16, name=f"fslab{n}_{b}",
                        tag="fslab", bufs=2,
                    )
                    nc.vector.tensor_scalar_mul(
                        slab[:], tap_src(b, n, 0, SP), ws[:, n:n + 1]
                    )
                    if dy != 0:
                        sv = slab[:].rearrange("p (i j) -> p i j", j=96)
                        jz = 0 if dy < 0 else 95
                        nc.vector.memset(sv[:, :, jz:jz + 1], 0.0)
                    slab_seq.append(n)
                    eng = nc.sync if len(slab_seq) % 2 else nc.scalar
                    eng.dma_start(out=y[n, b][:], in_=slab[:])

            # --- main pipeline
            for b in range(BPC):
                for k in range(NCK):
                    ps = pspool.tile([COUT, CK], F32, name=f"ps{b}_{k}", tag="ps")
                    nc.tensor.matmul(
                        ps[:], lhsT=ones64[:],
                        rhs=xts[b][:, k * CK:(k + 1) * CK],
                        start=True, stop=True,
                    )
                    nc.vector.tensor_copy(
                        s_e[b][:, HALO + k * CK:HALO + (k + 1) * CK], ps[:]
                    )
                    nc.scalar.copy(
                        s_o[b][:, HALO + 1 + k * CK:HALO + 1 + (k + 1) * CK], ps[:]
                    )
                    # each piece is emitted only after every cast it reads
                    # is already in the DVE stream (no reliance on the
                    # scheduler hoisting casts past blocked money ops).
                    # batch 0 starts with a small piece so output DMA
                    # begins as early as possible.
                    if b == 0 and k == 4:
                        emit_money(b, 0, HB // 2)
                    elif b == 0 and k == 9:
                        emit_money(b, HB // 2, HB // 2)
                    elif b == 1 and k == 9:
                        emit_money(b, 0, HB, sbufs=5)
                emit_money(b, HB, HB, sbufs=5)

    nc.finalize()
    return nc


_CACHE = {}


def _get_nc():
    if "nc" not in _CACHE:
        _CACHE["nc"] = _build_nc()
    return _CACHE["nc"]


def _run(x_full, w_full, **kwargs):
    nc = _get_nc()
    # W[n, c, o] -> [c, n*o] so the colsum matmul's lhsT ([c, o] slices) is
    # a plain contiguous SBUF tile.
    wt = np.ascontiguousarray(
        w_full.reshape(NTAP, COUT, COUT).transpose(1, 0, 2)
    ).reshape(COUT, NTAP * COUT).astype(ml_dtypes.bfloat16)
    # per core: [BPC, 9216, 64] -> [BPC, 64, 9216] bf16 (channel-major so the
    # PE ones-matmul contracts over the partition dim)
    xr = x_full.reshape(NCORES, BPC, SP, CIN)
    in_maps = [
        {
            "x": np.ascontiguousarray(
                xr[c].transpose(0, 2, 1).astype(ml_dtypes.bfloat16)
            ),
            "w": wt,
        }
        for c in range(NCORES)
    ]
    return run_bass_kernel_spmd(nc, in_maps, core_ids=list(range(NCORES)), **kwargs)


def _unshard(results):
    """Per-core y is [9, BPC, 128(o), 9216(f)]; full out wants [..., f, o]."""
    out = np.empty((3, 3, B, H, W_, COUT), np.float32)
    ov = out.reshape(NTAP, B, SP, COUT)
    for c, r in enumerate(results):
        yc = np.asarray(r["y"]).reshape(NTAP, BPC, COUT, SP)
        ov[:, BPC * c:BPC * (c + 1)] = yc.transpose(0, 1, 3, 2)
    return out


def kernel(**inputs):
    x_full = np.ascontiguousarray(np.asarray(inputs["inputs"], dtype=np.float32))
    w_full = np.ascontiguousarray(np.asarray(inputs["W"], dtype=np.float32))
    res = _run(x_full, w_full)
    return _unshard(res.results)


# revision 20
# speedup vs baseline: 1.1222x; 1.0190x over previous
"""Trainium2 Bass kernel for nn_ConvBundle_48146583388363.

Math: out[x,y,b,i,j,o] = s[b, i+x-1, j+y-1] * wsum[x,y,o]
  where s = inputs.sum(channel) (zero-padded at borders) and
  wsum = W.sum(axis=2).

Sharding: data-parallel over batch B=16 across 8 cores (2 batches/core).

Layout: cout (o=128) on SBUF partitions, flat per-batch spatial index
f = 96*i + j on the free dim. One PE matmul (ones[64,128].T @ x[64,f])
does the channel reduce AND broadcasts s[f] to all 128 partitions.
Each tap's shift is then just a free-dim AP offset into an s buffer
with 98-wide zero halos; each (tap, batch, half) output is a single
[128, 4608] tensor_scalar_mul with the per-partition scalar wsum[o],
which hits the DVE 4x perf mode (fp16, step-1, 4B-aligned, SBUF).

Two s copies at both alignment parities (s_e[98+f]=s[f] on DVE,
s_o[99+f]=s[f] on ACT, both cast f32->f16 from PSUM) make every tap
offset even so the 4x mode alignment requirement holds for all 9 taps.
Column-border zeros for the dy!=0 taps are strided memsets on the
output slab before DMA.

Output is written fp16 (halves the HBM write traffic, which is the
roofline: ~45 MB/core at ~358 GB/s). Host upcasts to f32 during
unshard; rel err ~1e-3 vs the 2e-2 gate.
"""

import ml_dtypes
import numpy as np

import concourse.bacc as bacc
import concourse.mybir as mybir
from concourse import tile
from concourse.bass_utils import run_bass_kernel_spmd

F32 = mybir.dt.float32
F16 = mybir.dt.float16
BF16 = mybir.dt.bfloat16

NCORES = 8
B, H, W_, CIN = 16, 96, 96, 64
COUT = 128
BPC = B // NCORES          # batches per core = 2
SP = H * W_                # 9216 spatial positions per batch
NTAP = 9
TAPS = [(x - 1, y - 1) for x in range(3) for y in range(3)]  # tap n = 3x+y
CK = 512                   # PSUM chunk (one bank) in f
NCK = SP // CK             # 18 chunks per batch
HALO = 98
L = HALO + SP + HALO       # s buffer length = 9412
HB = SP // 2               # money-op granularity = 4608


def _build_nc():
    nc = bacc.Bacc(None, target_bir_lowering=False)
    x = nc.dram_tensor("x", [BPC, CIN, SP], BF16, kind="ExternalInput")
    w = nc.dram_tensor("w", [COUT, NTAP * COUT], BF16, kind="ExternalInput")
    y = nc.dram_tensor("y", [NTAP, BPC, COUT, SP], F16, kind="ExternalOutput")

    with tile.TileContext(nc) as tc:
        with (
            tc.tile_pool(name="const", bufs=1) as cpool,
            tc.tile_pool(name="xin", bufs=1) as xpool,
            tc.tile_pool(name="psum_s", bufs=7, space="PSUM") as pspool,
            tc.tile_pool(name="psum_w", bufs=1, space="PSUM") as pwpool,
            tc.tile_pool(name="out", bufs=5) as opool,
        ):
            # --- input DMAs. W first (it gates wsum -> every money op);
            # batch 0 quartered across both HWDGE rings for an early start.
            w_sb = cpool.tile([COUT, NTAP * COUT], BF16, name="w_sb")
            nc.scalar.dma_start(out=w_sb[:], in_=w[:])
            xts = [xpool.tile([CIN, SP], BF16, name=f"xt{b}") for b in range(BPC)]
            QW = SP // 4
            for q, eng in enumerate([nc.sync, nc.scalar, nc.sync, nc.scalar]):
                eng.dma_start(
                    out=xts[0][:, q * QW:(q + 1) * QW], in_=x[0][:, q * QW:(q + 1) * QW]
                )
            for hlf, eng in enumerate([nc.sync, nc.scalar]):
                eng.dma_start(
                    out=xts[1][:, hlf * HB:(hlf + 1) * HB],
                    in_=x[1][:, hlf * HB:(hlf + 1) * HB],
                )

            # --- constants (no DMA: generated on-chip)
            ones64 = cpool.tile([CIN, COUT], BF16, name="ones64")
            nc.vector.memset(ones64[:], 1.0)
            onesc = cpool.tile([COUT, 1], BF16, name="onesc")
            nc.vector.memset(onesc[:], 1.0)

            # s buffers, both parities, zero halos
            s_e, s_o = [], []
            for b in range(BPC):
                se = cpool.tile([COUT, L], F16, name=f"s_e{b}")
                so = cpool.tile([COUT, L], F16, name=f"s_o{b}")
                nc.vector.memset(se[:, 0:HALO], 0.0)
                nc.vector.memset(se[:, HALO + SP:L], 0.0)
                nc.vector.memset(so[:, 0:HALO + 1], 0.0)
                nc.vector.memset(so[:, HALO + 1 + SP:L], 0.0)
                s_e.append(se)
                s_o.append(so)

            # --- wsum[o, n] = sum_c W[n, c, o]: 9 single-column matmuls.
            # ws lands via DVE (not ACT) so the s_o copy stream is never
            # head-of-line blocked behind the W dependency chain.
            pw = pwpool.tile([COUT, CK], F32, name="pw")
            for n in range(NTAP):
                nc.tensor.matmul(
                    pw[:, n:n + 1],
                    lhsT=w_sb[:, n * COUT:(n + 1) * COUT],
                    rhs=onesc[:],
                    start=True, stop=True, skip_group_check=True,
                )
            ws = cpool.tile([COUT, 16], F32, name="ws")
            nc.vector.tensor_copy(ws[:, 0:NTAP], pw[:, 0:NTAP])

            # tap read offsets (all even by construction)
            def tap_src(b, n, f0, fd):
                dx, dy = TAPS[n]
                d = 96 * dx + dy
                if dy == 0:
                    buf, st = s_e[b], HALO + d
                else:
                    buf, st = s_o[b], HALO + 1 + d
                return buf[:, st + f0: st + f0 + fd]

            slab_seq = []  # emission counter for DMA ring alternation

            def emit_money(b, f0, fd, sbufs=5):
                """One money op + border fix + DMA per tap for f range
                [f0, f0+fd). The slab tile covers exactly that range."""
                # dy==0 taps first: they only need s_e (DVE) chunks, which
                # land earlier than the ACT s_o chunks.
                for n in sorted(range(NTAP), key=lambda n: TAPS[n][1] != 0):
                    dx, dy = TAPS[n]
                    slab = opool.tile(
                        [COUT, fd], F16, name=f"slab{n}_{b}_{f0}",
                        tag=f"slab{fd}", bufs=sbufs,
                    )
                    nc.vector.tensor_scalar_mul(
                        slab[:], tap_src(b, n, f0, fd), ws[:, n:n + 1]
                    )
                    if dy != 0:
                        sv = slab[:].rearrange("p (i j) -> p i j", j=96)
                        jz = 0 if dy < 0 else 95
                        nc.vector.memset(sv[:, :, jz:jz + 1], 0.0)
                    slab_seq.append(n)
                    eng = nc.sync if len(slab_seq) % 2 else nc.scalar
                    eng.dma_start(
                        out=y[n, b][:, f0:f0 + fd], in_=slab[:]
                    )

            tap_order = sorted(range(NTAP), key=lambda n: TAPS[n][1] != 0)

            def emit_full(b):
                """Whole-batch money per tap [COUT, SP] -> one 2.25MB DMA
                with 18.4KB/partition contiguous runs. Emitted after all of
                batch b's casts; steady produce->drain keeps DMA fed."""
                for n in tap_order:
                    dx, dy = TAPS[n]
                    slab = opool.tile(
                        [COUT, SP], F16, name=f"fslab{n}_{b}",
                        tag="fslab", bufs=2,
                    )
                    nc.vector.tensor_scalar_mul(
                        slab[:], tap_src(b, n, 0, SP), ws[:, n:n + 1]
                    )
                    if dy != 0:
                        sv = slab[:].rearrange("p (i j) -> p i j", j=96)
                        jz = 0 if dy < 0 else 95
                        nc.vector.memset(sv[:, :, jz:jz + 1], 0.0)
                    slab_seq.append(n)
                    eng = nc.sync if len(slab_seq) % 2 else nc.scalar
                    eng.dma_start(out=y[n, b][:], in_=slab[:])

            # --- main pipeline
            for b in range(BPC):
                for k in range(NCK):
                    ps = pspool.tile([COUT, CK], F32, name=f"ps{b}_{k}", tag="ps")
                    nc.tensor.matmul(
                        ps[:], lhsT=ones64[:],
                        rhs=xts[b][:, k * CK:(k + 1) * CK],
                        start=True, stop=True,
                    )
                    nc.vector.tensor_copy(
                        s_e[b][:, HALO + k * CK:HALO + (k + 1) * CK], ps[:]
                    )
                    nc.scalar.copy(
                        s_o[b][:, HALO + 1 + k * CK:HALO + 1 + (k + 1) * CK], ps[:]
                    )
                    # each piece is emitted only after every cast it reads
                    # is already in the DVE stream (no reliance on the
                    # scheduler hoisting casts past blocked money ops).
                    # batch 0 starts with a small piece so output DMA
                    # begins as early as possible.
                    if b == 0 and k == 4:
                        emit_money(b, 0, HB // 2)
                    elif b == 0 and k == 9:
                        emit_money(b, HB // 2, HB // 2)
                    elif b == 1 and k == 9:
                        emit_money(b, 0, HB, sbufs=5)
                emit_money(b, HB, HB, sbufs=5)

    nc.finalize()
    return nc


_CACHE = {}


def _get_nc():
    if "nc" not in _CACHE:
        _CACHE["nc"] = _build_nc()
    return _CACHE["nc"]


def _run(x_full, w_full, **kwargs):
    nc = _get_nc()
    # W[n, c, o] -> [c, n*o] so the colsum matmul's lhsT ([c, o] slices) is
    # a plain contiguous SBUF tile.
    wt = np.ascontiguousarray(
        w_full.reshape(NTAP, COUT, COUT).transpose(1, 0, 2)
    ).reshape(COUT, NTAP * COUT).astype(ml_dtypes.bfloat16)
    # per core: [BPC, 9216, 64] -> [BPC, 64, 9216] bf16 (channel-major so the
    # PE ones-matmul contracts over the partition dim)
    xr = x_full.reshape(NCORES, BPC, SP, CIN)
    in_maps = [
        {
            "x": np.ascontiguousarray(
                xr[c].transpose(0, 2, 1).astype(ml_dtypes.bfloat16)
            ),
            "w": wt,
        }
        for c in range(NCORES)
    ]
    return run_bass_kernel_spmd(nc, in_maps, core_ids=list(range(NCORES)), **kwargs)


def _unshard(results):
    """Per-core y is [9, BPC, 128(o), 9216(f)]; full out wants [..., f, o]."""
    out = np.empty((3, 3, B, H, W_, COUT), np.float32)
    ov = out.reshape(NTAP, B, SP, COUT)
    for c, r in enumerate(results):
        yc = np.asarray(r["y"]).reshape(NTAP, BPC, COUT, SP)
        ov[:, BPC * c:BPC * (c + 1)] = yc.transpose(0, 1, 3, 2)
    return out


def kernel(**inputs):
    x_full = np.ascontiguousarray(np.asarray(inputs["inputs"], dtype=np.float32))
    w_full = np.ascontiguousarray(np.asarray(inputs["W"], dtype=np.float32))
    res = _run(x_full, w_full)
    return _unshard(res.results)
